# revision 13
# baseline (speedup 1.0000x reference)
"""Multi-head causal attention (B=2, S=2048, D=1024, H=16) on 8 trn2 cores.

Sharding: core c handles batch b = c // 4 and head group g = c % 4 (4 heads,
256 feature columns). Each core computes its heads' attention context and a
partial output projection (ctx_g @ Wo[rows_g]); the host sums the 4 partials
per batch and adds bo.

v2 layout (all matmul operands bf16, fp32 PSUM accumulate):
- x is host-transposed to xT [D, S] bf16 so the QKV contraction dim sits on
  SBUF partitions; Q^T/K^T are produced head-major ([64*(h%2), h//2] rows) so
  score matmuls contract 64 partitions with matching base partitions and no
  transposes; P^T = exp(S^T) is directly the moving operand of the PV matmul.
- Attention is sq-half-major: for each half (sq 0:1024 / 1024:2048), per
  (head, sk-tile) ONE wide score matmul + ONE wide exp + ONE wide PV matmul
  into a per-head [128, 1024] ctx psum holding 2 sq-tiles' chains (per-mm
  stop + skip_group_check; windows of finished sq-tiles are never rewritten,
  so eager normalize reads are race-free under subtile dep tracking).
- Softmax denominator comes from a ones column folded into V; normalization
  happens at the ctx psum->sbuf copy: DVE reciprocal of the denom row,
  gpsimd partition_broadcast (no DRAM round trip), DVE multiply to bf16.
- Emission interleaves projection slice 1 into half-0 attention and the
  output projection into half-1 attention so the PE stays busy while the
  scalar engine streams the exps (the ~90us serial floor of this kernel).
"""

import os
import sys
import types
from contextlib import ExitStack

import numpy as np
import ml_dtypes

import concourse.bacc as bacc
import concourse.bass as bass
import concourse.mybir as mybir
import concourse.tile as tile
from concourse.bass_utils import run_bass_kernel_spmd


def _install_ntff_hook():
    """The agent image's antenv lacks axon_hooks, so trn_boot's NTFF hook
    install degrades silently. Recreate the module + hook so trace=True works."""
    if "antenv.axon_hooks" in sys.modules:
        return
    try:
        mod = types.ModuleType("antenv.axon_hooks")
        holder = [None]
        mod.set_axon_ntff_profile_hook = lambda h: holder.__setitem__(0, h)
        mod.get_axon_ntff_profile_hook = lambda: holder[0]
        from trn_agent_boot.trn_boot import _ntff_profile_via_ctypes

        hook = _ntff_profile_via_ctypes("/opt/axon/libaxon_pjrt.so")
        if hook is None:
            return
        mod.set_axon_ntff_profile_hook(hook)
        sys.modules["antenv.axon_hooks"] = mod
    except Exception:
        pass


B, S, D, H, HD = 2, 2048, 1024, 16, 64
NCORES = 8
GROUPS = 4          # head groups (cores) per batch
HC = H // GROUPS    # heads per core
DG = HC * HD        # feature columns per core (256)
P = 128
KSUB = D // P       # 8 contraction subtiles for the projections
SLC = 1024          # projection s-slice width
NST = S // P        # 16 sk subtiles of 128
F32 = mybir.dt.float32
BF16 = mybir.dt.bfloat16
BF16NP = ml_dtypes.bfloat16

_CACHE = {}


class _MHA:
    """Holds the tile handles so emission helpers can be interleaved freely."""

    def __init__(self, tc, ctx, xT, wq, wk, wv, wo, out):
        self.tc = tc
        self.nc = tc.nc
        self.out = out
        self.scale = 1.0 / float(np.sqrt(np.float32(HD)))
        nc = self.nc

        self.consts = ctx.enter_context(tc.tile_pool(name="consts", bufs=1))
        # PSUM: sps = scores / projections / out-projection [128,1024] (2 banks
        # x2), cps = per-head ctx accumulators [128,1024] (2 banks x2)
        self.sps = ctx.enter_context(tc.tile_pool(name="sps", bufs=2, space="PSUM"))
        self.cps = ctx.enter_context(tc.tile_pool(name="cps", bufs=2, space="PSUM"))
        self.xw = ctx.enter_context(tc.tile_pool(name="xw", bufs=2))
        self.ptp = ctx.enter_context(tc.tile_pool(name="ptp", bufs=4))
        self.smalls = ctx.enter_context(tc.tile_pool(name="smalls", bufs=4))
        self.bcp = ctx.enter_context(tc.tile_pool(name="bcp", bufs=3))
        self.outp = ctx.enter_context(tc.tile_pool(name="outp", bufs=3))

        c = self.consts
        self.qt = c.tile([P, 2, S], BF16)    # head h rows at [64*(h%2), h//2]
        self.kt = c.tile([P, 2, S], BF16)
        self.vt = c.tile([P, NST, HC, P], BF16)  # [sk, sst, h, 64v+1+63pad]
        self.ctxt = c.tile([P, 2, S], BF16)  # normalized ctx^T, qt layout
        self.wq_sb = c.tile([P, KSUB, DG], BF16)
        self.wk_sb = c.tile([P, KSUB, DG], BF16)
        self.wv_sb = c.tile([P, KSUB, DG], BF16)
        self.wo_sb = c.tile([P, DG // P, D], BF16)
        nc.sync.dma_start(out=self.wq_sb, in_=wq)
        nc.sync.dma_start(out=self.wk_sb, in_=wk)
        nc.sync.dma_start(out=self.wv_sb, in_=wv)
        nc.sync.dma_start(out=self.wo_sb, in_=wo)

        # V pad columns must be zero only to keep sim/HW psum garbage finite;
        # cheap one-off on gpsimd, off the DVE critical path.
        nc.gpsimd.memset(self.vt, 0.0)
        osc = c.tile([P, 1], F32)
        nc.vector.memset(osc, 1.0)
        for h in range(HC):
            ones_col = 64 if h % 2 == 0 else 0
            nc.vector.tensor_copy(
                out=self.vt[:, :, h, ones_col : ones_col + 1],
                in_=osc[:, None, :].to_broadcast((P, NST, 1)),
            )

        self.xT = xT
        self.xn = [None, None]
        self.cpsum = {}  # (h, half) -> psum tile
        self.pending_norm = []

    # ---- projections -------------------------------------------------
    def dma_x(self, n):
        xn = self.xw.tile([P, KSUB, SLC], BF16, tag="xn", name=f"xn_{n}")
        for k in range(KSUB):
            self.nc.sync.dma_start(
                out=xn[:, k, :], in_=self.xT[k * P : (k + 1) * P, n * SLC : (n + 1) * SLC]
            )
        self.xn[n] = xn

    def proj_qk(self, n, m, which):
        """One [128,1024] psum of Q^T or K^T for slice n, head-pair m.
        Matmul outputs may not cross a PSUM bank, so 512-col segments."""
        nc = self.nc
        w = self.wq_sb if which == "q" else self.wk_sb
        dst = self.qt if which == "q" else self.kt
        ps = self.sps.tile([P, SLC], F32, tag="ps", name=f"{which}ps_{n}_{m}")
        for half in range(2):
            for k in range(KSUB):
                nc.tensor.matmul(
                    ps[:, half * 512 : (half + 1) * 512],
                    lhsT=w[:, k, m * P : (m + 1) * P],
                    rhs=self.xn[n][:, k, half * 512 : (half + 1) * 512],
                    start=(k == 0),
                    stop=(k == KSUB - 1),
                )
        nc.vector.tensor_copy(out=dst[:, m, n * SLC : (n + 1) * SLC], in_=ps)

    def proj_v(self, n, a):
        """V for s-subtiles 8n+4a .. 8n+4a+3 -> vt (64 rows + ones layout)."""
        nc = self.nc
        ps = self.sps.tile([P, SLC], F32, tag="ps", name=f"vps_{n}_{a}")
        for j in range(4):
            for k in range(KSUB):
                nc.tensor.matmul(
                    ps[:, j * DG : (j + 1) * DG],
                    lhsT=self.xn[n][:, k, (4 * a + j) * P : (4 * a + j + 1) * P],
                    rhs=self.wv_sb[:, k, :],
                    start=(k == 0),
                    stop=(k == KSUB - 1),
                )
        st0 = 8 * n + 4 * a
        psv = ps.rearrange("p (t h d) -> p t h d", h=HC, d=HD)
        nc.vector.tensor_copy(
            out=self.vt[:, st0 : st0 + 4, 0:HC:2, 0:HD], in_=psv[:, :, 0:HC:2, :]
        )
        nc.vector.tensor_copy(
            out=self.vt[:, st0 : st0 + 4, 1:HC:2, HD:P], in_=psv[:, :, 1:HC:2, :]
        )

    # ---- attention ---------------------------------------------------
    def att_step(self, h, half, ski):
        """One (head, sk-tile) step within an sq half: wide score matmul,
        wide exp, diagonal mask, wide PV accumulate; normalize finished
        sq-tiles eagerly."""
        nc = self.nc
        hp = 64 * (h % 2)
        hm = h // 2
        g0 = half * SLC            # global sq start of this half
        a = max(g0, P * ski)       # global sq start of the valid window
        w = g0 + SLC - a           # window width
        if w <= 0:
            return
        diag = P * ski >= g0       # diagonal block lives in this half

        if (h, half) not in self.cpsum:
            self.cpsum[(h, half)] = self.cps.tile(
                [P, SLC], F32, tag="ctx", name=f"ctx_{h}_{half}"
            )
        cpsum = self.cpsum[(h, half)]

        l0 = a - g0  # local window start within the [128,1024] half tiles
        segs = [s for s in ((l0, min(512, SLC)), (max(l0, 512), SLC)) if s[0] < s[1]]

        spsum = self.sps.tile([P, SLC], F32, tag="ps", name=f"s_{h}_{half}_{ski}")
        for s0, s1 in segs:
            nc.tensor.matmul(
                spsum[:, s0:s1],
                lhsT=self.kt[hp : hp + 64, hm, ski * P : (ski + 1) * P],
                rhs=self.qt[hp : hp + 64, hm, g0 + s0 : g0 + s1],
                start=True,
                stop=True,
            )
        pt = self.ptp.tile([P, SLC], BF16, tag="pt", name=f"pt_{h}_{half}_{ski}")
        nc.scalar.activation(
            out=pt[:, 0:w],
            in_=spsum[:, l0:SLC],
            func=mybir.ActivationFunctionType.Exp,
            bias=0.0,
            scale=float(self.scale),
        )
        if diag:
            nc.gpsimd.affine_select(
                out=pt[:, 0:P],
                in_=pt[:, 0:P],
                pattern=[[1, P]],
                compare_op=mybir.AluOpType.is_ge,
                fill=0.0,
                base=0,
                channel_multiplier=-1,
            )
        for s0, s1 in segs:
            sqt = 2 * half + (s0 >= 512)
            nc.tensor.matmul(
                cpsum[:, s0:s1],
                lhsT=self.vt[:, ski, h, :],
                rhs=pt[:, s0 - l0 : s1 - l0],
                start=(ski == 0),
                stop=(ski == 4 * sqt + 3),
            )
        # eager normalize: sq-tile sqt finishes its chain at ski == 4*sqt+3;
        # its columns are never rewritten afterwards. Phase 1 (tiny DMA of the
        # denominator row to SBUF) is issued at the chain stop; phase 2 is
        # flushed later so the gpsimd broadcast never blocks its queue
        # waiting on the DMA.
        if ski % 4 == 3:
            sqt = ski // 4
            if sqt in (2 * half, 2 * half + 1):
                self.norm_start(h, half, sqt)

    def norm_start(self, h, half, sqt):
        nc = self.nc
        denom_row = 64 if h % 2 == 0 else 0
        cpsum = self.cpsum[(h, half)]
        l0 = sqt * 512 - half * SLC    # local window start in the half tile
        den = self.smalls.tile([1, 512], F32, tag="den", name=f"den_{h}_{sqt}")
        nc.vector.tensor_copy(out=den, in_=cpsum[denom_row : denom_row + 1, l0 : l0 + 512])
        self.pending_norm.append((h, half, sqt, den))

    def norm_flush(self):
        nc = self.nc
        for h, half, sqt, den in self.pending_norm:
            hp = 64 * (h % 2)
            hm = h // 2
            ctx_rows = 0 if h % 2 == 0 else 64
            cpsum = self.cpsum[(h, half)]
            l0 = sqt * 512 - half * SLC
            sq0 = sqt * 512
            bc = self.bcp.tile([P, 512], F32, tag="bc", name=f"bc_{h}_{sqt}")
            nc.gpsimd.partition_broadcast(bc, den, channels=P)
            nc.vector.reciprocal(out=bc, in_=bc)
            nc.vector.tensor_tensor(
                self.ctxt[hp : hp + 64, hm, sq0 : sq0 + 512],
                cpsum[ctx_rows : ctx_rows + 64, l0 : l0 + 512],
                bc[ctx_rows : ctx_rows + 64, :],
                mybir.AluOpType.mult,
            )
        self.pending_norm = []

    # ---- output projection (tail only: attention ctx psums are free) --
    def outproj(self, st):
        nc = self.nc
        ps = self.cps.tile([P, D], F32, tag="ctx", name=f"ops_{st}")
        for nn in range(2):
            for k in range(DG // P):
                nc.tensor.matmul(
                    ps[:, nn * 512 : (nn + 1) * 512],
                    lhsT=self.ctxt[:, k, st * P : (st + 1) * P],
                    rhs=self.wo_sb[:, k, nn * 512 : (nn + 1) * 512],
                    start=(k == 0),
                    stop=(k == DG // P - 1),
                )
        ot = self.outp.tile([P, D], F32, tag="ot", name=f"ot_{st}")
        nc.vector.tensor_copy(out=ot, in_=ps)
        nc.sync.dma_start(out=self.out[st * P : (st + 1) * P, :], in_=ot)


def _emit(tc, xT, wq, wk, wv, wo, out):
    with ExitStack() as ctx:
        m = _MHA(tc, ctx, xT, wq, wk, wv, wo, out)

        m.dma_x(0)
        m.proj_qk(0, 0, "q")
        m.proj_qk(0, 0, "k")
        m.proj_v(0, 0)
        m.proj_v(0, 1)
        m.dma_x(1)

        # half 0: heads 0..3 over ski 0..7; the remaining projection units
        # are emitted at head boundaries (PE filler while ACT streams exps).
        # norm_flush points lag one sq-tile behind the denominator DMA.
        boundary = {
            0: [lambda: m.proj_qk(0, 1, "q"), lambda: m.proj_qk(0, 1, "k")],
            1: [lambda: m.proj_qk(1, 0, "q"), lambda: m.proj_qk(1, 0, "k")],
            2: [
                lambda: m.proj_v(1, 0),
                lambda: m.proj_v(1, 1),
                lambda: m.proj_qk(1, 1, "q"),
                lambda: m.proj_qk(1, 1, "k"),
            ],
        }
        for h in range(HC):
            for ski in range(8):
                m.att_step(h, 0, ski)
                if ski % 4 == 1:
                    m.norm_flush()
            m.norm_flush()
            for unit in boundary.get(h, []):
                unit()

        # half 1: heads 0..3 over ski 0..15
        for h in range(HC):
            for ski in range(16):
                m.att_step(h, 1, ski)
                if ski % 4 == 1:
                    m.norm_flush()
            m.norm_flush()

        # out-projection tail: PE-only phase, ctx psum banks are free
        for st in range(NST):
            m.outproj(st)


def build_nc():
    if "nc" in _CACHE:
        return _CACHE["nc"]
    nc = bacc.Bacc("TRN2", target_bir_lowering=False, debug=False, num_devices=NCORES)
    xT = nc.dram_tensor("xT", (D, S), BF16, kind="ExternalInput").ap()
    wq = nc.dram_tensor("wq", (P, KSUB, DG), BF16, kind="ExternalInput").ap()
    wk = nc.dram_tensor("wk", (P, KSUB, DG), BF16, kind="ExternalInput").ap()
    wv = nc.dram_tensor("wv", (P, KSUB, DG), BF16, kind="ExternalInput").ap()
    wo = nc.dram_tensor("wo", (P, DG // P, D), BF16, kind="ExternalInput").ap()
    out = nc.dram_tensor("out", (S, D), F32, kind="ExternalOutput").ap()
    with tile.TileContext(nc) as tc:
        _emit(tc, xT, wq, wk, wv, wo, out)
    nc.compile()
    _CACHE["nc"] = nc
    return nc


def make_in_maps(x, Wq, Wk, Wv, Wo):
    x = np.asarray(x, np.float32)
    in_maps = []
    for c in range(NCORES):
        b, g = c // GROUPS, c % GROUPS
        cols = slice(g * DG, (g + 1) * DG)

        def wslice(W):
            # [D, DG] -> [128, KSUB, DG] with [p, k, m] = W[k*128+p, m]
            return np.ascontiguousarray(
                np.asarray(W, np.float32)[:, cols]
                .reshape(KSUB, P, DG)
                .transpose(1, 0, 2)
                .astype(BF16NP)
            )

        wo_c = np.ascontiguousarray(
            np.asarray(Wo, np.float32)[cols, :]
            .reshape(DG // P, P, D)
            .transpose(1, 0, 2)
            .astype(BF16NP)
        )
        in_maps.append(
            {
                "xT": np.ascontiguousarray(x[b].T.astype(BF16NP)),
                "wq": wslice(Wq),
                "wk": wslice(Wk),
                "wv": wslice(Wv),
                "wo": wo_c,
            }
        )
    return in_maps


def kernel(x, Wq, Wk, Wv, Wo, bo):
    nc = build_nc()
    in_maps = make_in_maps(x, Wq, Wk, Wv, Wo)
    trace = bool(int(os.environ.get("MHA_TRACE", "0")))
    if trace:
        _install_ntff_hook()
    res = run_bass_kernel_spmd(
        nc, in_maps, core_ids=list(range(NCORES)), trace=trace,
        trace_cores=list(range(NCORES)) if trace else None,
    )
    _CACHE["last_results"] = res
    bo = np.asarray(bo, np.float32)
    out = np.zeros((B, S, D), np.float32)
    for c in range(NCORES):
        out[c // GROUPS] += res.results[c]["out"]
    out += bo[None, None, :]
    return out


# revision 17
# speedup vs baseline: 1.2788x; 1.2788x over previous
"""Multi-head causal attention (B=2, S=2048, D=1024, H=16) on 8 trn2 cores.

Sharding: core c handles batch b = c // 4 and head group g = c % 4 (4 heads,
256 feature columns). Each core computes its heads' attention context and a
partial output projection (ctx_g @ Wo[rows_g]); the host sums the 4 partials
per batch (upcasting the bf16 partials) and adds bo.

v4 layout (all matmul operands bf16, fp32 PSUM accumulate):
- x is host-transposed to xT [D, S] bf16 so the QKV contraction dim sits on
  SBUF partitions; Q^T/K^T are produced head-major (head h on partitions
  [64*(h%2):+64] of tile m=h//2) so score matmuls contract 64 partitions with
  matching base partitions and no transposes; P^T = exp(S^T) feeds the PV
  matmul directly as the moving operand.
- Attention is sq-half-major (sq halves of 1024): per (head, sk-tile) one
  [128,1024] score psum (matmuls segmented at 512 -- PSUM bank limit), ONE
  wide exp (bf16 out), gpsimd affine_select on the diagonal block, PV
  accumulated into per-(head,half,512-seg) ctx psums with exact chain flags.
- Softmax denominator: ones column folded into V; normalize at the ctx
  psum->sbuf copy via DVE row copy + gpsimd partition_broadcast +
  reciprocal_approx_fast (no DRAM round trip, no slow full reciprocal);
  two-phase emission keeps the broadcast from stalling the gpsimd queue.
- PSUM: scores 2x[128,1024] (4 banks) + ctx 3x[128,512] (3) + filler
  1x[128,512] (1). The filler bank runs projection / out-projection chains
  dripped one per attention step, so the PE always has work while the scalar
  engine streams the ~80us of exps -- keeping the PE p-state at full clock.
"""

import os
import sys
import types
from contextlib import ExitStack

import numpy as np
import ml_dtypes

import concourse.bacc as bacc
import concourse.bass as bass
import concourse.mybir as mybir
import concourse.tile as tile
from concourse.bass_utils import run_bass_kernel_spmd


def _install_ntff_hook():
    """The agent image's antenv lacks axon_hooks, so trn_boot's NTFF hook
    install degrades silently. Recreate the module + hook so trace=True works."""
    if "antenv.axon_hooks" in sys.modules:
        return
    try:
        mod = types.ModuleType("antenv.axon_hooks")
        holder = [None]
        mod.set_axon_ntff_profile_hook = lambda h: holder.__setitem__(0, h)
        mod.get_axon_ntff_profile_hook = lambda: holder[0]
        from trn_agent_boot.trn_boot import _ntff_profile_via_ctypes

        hook = _ntff_profile_via_ctypes("/opt/axon/libaxon_pjrt.so")
        if hook is None:
            return
        mod.set_axon_ntff_profile_hook(hook)
        sys.modules["antenv.axon_hooks"] = mod
    except Exception:
        pass


B, S, D, H, HD = 2, 2048, 1024, 16, 64
NCORES = 8
GROUPS = 4          # head groups (cores) per batch
HC = H // GROUPS    # heads per core
DG = HC * HD        # feature columns per core (256)
P = 128
KSUB = D // P       # 8 contraction subtiles for the projections
SLC = 1024          # projection s-slice / attention half width
NST = S // P        # 16 sk subtiles of 128
F32 = mybir.dt.float32
BF16 = mybir.dt.bfloat16
BF16NP = ml_dtypes.bfloat16

_CACHE = {}


class _MHA:
    """Holds tile handles so emission helpers can be interleaved freely."""

    def __init__(self, tc, ctx, xT, wq, wk, wv, wo, out):
        self.tc = tc
        self.nc = tc.nc
        self.out = out
        self.scale = 1.0 / float(np.sqrt(np.float32(HD)))
        nc = self.nc

        self.consts = ctx.enter_context(tc.tile_pool(name="consts", bufs=1))
        self.sps = ctx.enter_context(tc.tile_pool(name="sps", bufs=2, space="PSUM"))
        self.cps = ctx.enter_context(tc.tile_pool(name="cps", bufs=3, space="PSUM"))
        self.fps = ctx.enter_context(tc.tile_pool(name="fps", bufs=1, space="PSUM"))
        self.xw = ctx.enter_context(tc.tile_pool(name="xw", bufs=2))
        self.ptp = ctx.enter_context(tc.tile_pool(name="ptp", bufs=4))
        self.smalls = ctx.enter_context(tc.tile_pool(name="smalls", bufs=4))
        self.bcp = ctx.enter_context(tc.tile_pool(name="bcp", bufs=3))
        self.outp = ctx.enter_context(tc.tile_pool(name="outp", bufs=3))

        c = self.consts
        self.qt = c.tile([P, 2, S], BF16)    # head h rows at [64*(h%2), h//2]
        self.kt = c.tile([P, 2, S], BF16)
        self.vt = c.tile([P, NST, HC, P], BF16)  # [sk, sst, h, 64v+1+63pad]
        self.ctxt = c.tile([P, 2, S], BF16)  # normalized ctx^T, qt layout
        self.wq_sb = c.tile([P, KSUB, DG], BF16)
        self.wk_sb = c.tile([P, KSUB, DG], BF16)
        self.wv_sb = c.tile([P, KSUB, DG], BF16)
        self.wo_sb = c.tile([P, DG // P, D], BF16)
        nc.sync.dma_start(out=self.wq_sb, in_=wq)
        nc.sync.dma_start(out=self.wk_sb, in_=wk)
        nc.sync.dma_start(out=self.wv_sb, in_=wv)
        nc.sync.dma_start(out=self.wo_sb, in_=wo)

        # V pad columns must be zero (psum garbage must stay finite); one-off
        # on gpsimd, off the DVE critical path.
        nc.gpsimd.memset(self.vt, 0.0)
        osc = c.tile([P, 1], F32)
        nc.vector.memset(osc, 1.0)
        for h in range(HC):
            ones_col = 64 if h % 2 == 0 else 0
            nc.vector.tensor_copy(
                out=self.vt[:, :, h, ones_col : ones_col + 1],
                in_=osc[:, None, :].to_broadcast((P, NST, 1)),
            )

        self.xT = xT
        self.xn = [None, None]
        self.cpsum = {}  # (h, half, seg) -> [128,512] psum tile
        self.pending_norm = []

    # ---- projections (filler units on the 1-bank fps pool) ------------
    def dma_x(self, n):
        xn = self.xw.tile([P, KSUB, SLC], BF16, tag="xn", name=f"xn_{n}")
        for k in range(KSUB):
            self.nc.sync.dma_start(
                out=xn[:, k, :], in_=self.xT[k * P : (k + 1) * P, n * SLC : (n + 1) * SLC]
            )
        self.xn[n] = xn

    def proj_qk(self, n, m, which, half):
        """One [128,512] chain of Q^T or K^T: slice n, head-pair m, 512-half."""
        nc = self.nc
        w = self.wq_sb if which == "q" else self.wk_sb
        dst = self.qt if which == "q" else self.kt
        ps = self.fps.tile([P, 512], F32, tag="f", name=f"{which}ps_{n}_{m}_{half}")
        for k in range(KSUB):
            nc.tensor.matmul(
                ps,
                lhsT=w[:, k, m * P : (m + 1) * P],
                rhs=self.xn[n][:, k, half * 512 : (half + 1) * 512],
                start=(k == 0),
                stop=(k == KSUB - 1),
            )
        c0 = n * SLC + half * 512
        nc.vector.tensor_copy(out=dst[:, m, c0 : c0 + 512], in_=ps)

    def proj_v(self, n, j2):
        """V for s-subtiles (8n+2*j2, 8n+2*j2+1) -> vt rows with ones layout."""
        nc = self.nc
        ps = self.fps.tile([P, 512], F32, tag="f", name=f"vps_{n}_{j2}")
        for j in range(2):
            for k in range(KSUB):
                nc.tensor.matmul(
                    ps[:, j * DG : (j + 1) * DG],
                    lhsT=self.xn[n][:, k, (2 * j2 + j) * P : (2 * j2 + j + 1) * P],
                    rhs=self.wv_sb[:, k, :],
                    start=(k == 0),
                    stop=(k == KSUB - 1),
                )
        st0 = 8 * n + 2 * j2
        psv = ps.rearrange("p (t h d) -> p t h d", h=HC, d=HD)
        nc.vector.tensor_copy(
            out=self.vt[:, st0 : st0 + 2, 0:HC:2, 0:HD], in_=psv[:, :, 0:HC:2, :]
        )
        nc.vector.tensor_copy(
            out=self.vt[:, st0 : st0 + 2, 1:HC:2, HD:P], in_=psv[:, :, 1:HC:2, :]
        )

    # ---- attention ----------------------------------------------------
    def att_step(self, h, half, ski):
        """One (head, sk-tile) step in an sq half: segmented score matmuls,
        one wide exp, diagonal mask, segmented PV accumulate."""
        nc = self.nc
        hp = 64 * (h % 2)
        hm = h // 2
        g0 = half * SLC            # global sq start of this half
        a = max(g0, P * ski)       # global sq start of the valid window
        w = g0 + SLC - a
        if w <= 0:
            return
        diag = P * ski >= g0
        l0 = a - g0
        segs = [s for s in ((l0, min(512, SLC)), (max(l0, 512), SLC)) if s[0] < s[1]]

        spsum = self.sps.tile([P, SLC], F32, tag="ps", name=f"s_{h}_{half}_{ski}")
        for s0, s1 in segs:
            nc.tensor.matmul(
                spsum[:, s0:s1],
                lhsT=self.kt[hp : hp + 64, hm, ski * P : (ski + 1) * P],
                rhs=self.qt[hp : hp + 64, hm, g0 + s0 : g0 + s1],
                start=True,
                stop=True,
            )
        pt = self.ptp.tile([P, SLC], BF16, tag="pt", name=f"pt_{h}_{half}_{ski}")
        nc.scalar.activation(
            out=pt[:, 0:w],
            in_=spsum[:, l0:SLC],
            func=mybir.ActivationFunctionType.Exp,
            bias=0.0,
            scale=float(self.scale),
        )
        if diag:
            nc.gpsimd.affine_select(
                out=pt[:, 0:P],
                in_=pt[:, 0:P],
                pattern=[[1, P]],
                compare_op=mybir.AluOpType.is_ge,
                fill=0.0,
                base=0,
                channel_multiplier=-1,
            )
        for s0, s1 in segs:
            seg = s0 >= 512
            sqt = 2 * half + seg
            key = (h, half, seg)
            if key not in self.cpsum:
                self.cpsum[key] = self.cps.tile(
                    [P, 512], F32, tag="ctx", name=f"ctx_{h}_{half}_{seg}"
                )
            nc.tensor.matmul(
                self.cpsum[key][:, s0 - 512 * seg : s1 - 512 * seg],
                lhsT=self.vt[:, ski, h, :],
                rhs=pt[:, s0 - l0 : s1 - l0],
                start=(ski == 0),
                stop=(ski == 4 * sqt + 3),
            )
        if ski % 4 == 3:
            sqt = ski // 4
            if sqt in (2 * half, 2 * half + 1):
                self.norm_start(h, half, sqt)

    def norm_start(self, h, half, sqt):
        """Phase 1 of the eager normalize: pull the denominator row out of
        the finished ctx psum. Phase 2 is flushed later (one sq-tile lag)."""
        nc = self.nc
        denom_row = 64 if h % 2 == 0 else 0
        cpsum = self.cpsum[(h, half, sqt % 2)]
        den = self.smalls.tile([1, 512], F32, tag="den", name=f"den_{h}_{sqt}")
        nc.vector.tensor_copy(out=den, in_=cpsum[denom_row : denom_row + 1, :])
        self.pending_norm.append((h, half, sqt, den))

    def norm_flush(self):
        nc = self.nc
        for h, half, sqt, den in self.pending_norm:
            hp = 64 * (h % 2)
            hm = h // 2
            ctx_rows = 0 if h % 2 == 0 else 64
            cpsum = self.cpsum[(h, half, sqt % 2)]
            sq0 = sqt * 512
            bc = self.bcp.tile([P, 512], F32, tag="bc", name=f"bc_{h}_{sqt}")
            nc.gpsimd.partition_broadcast(bc, den, channels=P)
            nc.vector.reciprocal_approx_fast(out=bc, in_=bc)
            nc.vector.tensor_tensor(
                self.ctxt[hp : hp + 64, hm, sq0 : sq0 + 512],
                cpsum[ctx_rows : ctx_rows + 64, :],
                bc[ctx_rows : ctx_rows + 64, :],
                mybir.AluOpType.mult,
            )
        self.pending_norm = []

    # ---- output projection (filler units) -----------------------------
    def outproj(self, st, nn):
        nc = self.nc
        ps = self.fps.tile([P, 512], F32, tag="f", name=f"ops_{st}_{nn}")
        for k in range(DG // P):
            nc.tensor.matmul(
                ps,
                lhsT=self.ctxt[:, k, st * P : (st + 1) * P],
                rhs=self.wo_sb[:, k, nn * 512 : (nn + 1) * 512],
                start=(k == 0),
                stop=(k == DG // P - 1),
            )
        ot = self.outp.tile([P, 512], BF16, tag="ot", name=f"ot_{st}_{nn}")
        nc.vector.tensor_copy(out=ot, in_=ps)
        nc.sync.dma_start(
            out=self.out[st * P : (st + 1) * P, nn * 512 : (nn + 1) * 512], in_=ot
        )


def _emit(tc, xT, wq, wk, wv, wo, out):
    with ExitStack() as ctx:
        m = _MHA(tc, ctx, xT, wq, wk, wv, wo, out)

        # Minimal lead-in: x slice 0, then only the Q/K tiles head 0 needs.
        m.dma_x(0)
        m.proj_qk(0, 0, "q", 0)
        m.proj_qk(0, 0, "q", 1)
        m.proj_qk(0, 0, "k", 0)
        m.proj_qk(0, 0, "k", 1)
        m.dma_x(1)

        # Remaining projection work becomes filler units, dripped one per
        # attention step so the PE never idles while ACT streams the exps.
        # V units for slice 0 MUST be emitted before the h0 att_step that
        # first consumes them (PV of ski needs vt s-subtile ski).
        fill = []
        fill += [lambda hf=hf: m.proj_qk(0, 1, "q", hf) for hf in range(2)]
        fill += [lambda hf=hf: m.proj_qk(0, 1, "k", hf) for hf in range(2)]
        fill += [lambda hf=hf: m.proj_qk(1, 0, "q", hf) for hf in range(2)]
        fill += [lambda hf=hf: m.proj_qk(1, 0, "k", hf) for hf in range(2)]
        fill += [lambda j2=j2: m.proj_v(1, j2) for j2 in range(4)]
        fill += [lambda hf=hf: m.proj_qk(1, 1, "q", hf) for hf in range(2)]
        fill += [lambda hf=hf: m.proj_qk(1, 1, "k", hf) for hf in range(2)]
        fi = 0

        # half 0: heads 0..3 over ski 0..7
        for h in range(HC):
            for ski in range(8):
                if h == 0 and ski % 2 == 0:
                    m.proj_v(0, ski // 2)   # covers s-subtiles ski, ski+1
                m.att_step(h, 0, ski)
                if h >= 1 and fi < len(fill):
                    fill[fi](); fi += 1
                if ski % 4 == 1:
                    m.norm_flush()
            m.norm_flush()
        while fi < len(fill):
            fill[fi](); fi += 1

        # half 1: heads 0..3 over ski 0..15; out-projection of the finished
        # first half (st 0..7) becomes the filler stream.
        ops = [(st, nn) for st in range(8) for nn in range(2)]
        oi = 0
        for h in range(HC):
            for ski in range(16):
                m.att_step(h, 1, ski)
                if h >= 1 and oi < len(ops):
                    st, nn = ops[oi]; m.outproj(st, nn); oi += 1
                if ski % 4 == 1:
                    m.norm_flush()
            m.norm_flush()
        while oi < len(ops):
            st, nn = ops[oi]; m.outproj(st, nn); oi += 1

        # tail: st 8..15 (st 12..15 need h3's last normalize)
        for st in range(8, NST):
            for nn in range(2):
                m.outproj(st, nn)


def build_nc():
    if "nc" in _CACHE:
        return _CACHE["nc"]
    nc = bacc.Bacc("TRN2", target_bir_lowering=False, debug=False, num_devices=NCORES)
    xT = nc.dram_tensor("xT", (D, S), BF16, kind="ExternalInput").ap()
    wq = nc.dram_tensor("wq", (P, KSUB, DG), BF16, kind="ExternalInput").ap()
    wk = nc.dram_tensor("wk", (P, KSUB, DG), BF16, kind="ExternalInput").ap()
    wv = nc.dram_tensor("wv", (P, KSUB, DG), BF16, kind="ExternalInput").ap()
    wo = nc.dram_tensor("wo", (P, DG // P, D), BF16, kind="ExternalInput").ap()
    out = nc.dram_tensor("out", (S, D), BF16, kind="ExternalOutput").ap()
    with tile.TileContext(nc) as tc:
        _emit(tc, xT, wq, wk, wv, wo, out)
    nc.compile()
    _CACHE["nc"] = nc
    return nc


def make_in_maps(x, Wq, Wk, Wv, Wo):
    x = np.asarray(x, np.float32)
    in_maps = []
    for c in range(NCORES):
        b, g = c // GROUPS, c % GROUPS
        cols = slice(g * DG, (g + 1) * DG)

        def wslice(W):
            # [D, DG] -> [128, KSUB, DG] with [p, k, m] = W[k*128+p, m]
            return np.ascontiguousarray(
                np.asarray(W, np.float32)[:, cols]
                .reshape(KSUB, P, DG)
                .transpose(1, 0, 2)
                .astype(BF16NP)
            )

        wo_c = np.ascontiguousarray(
            np.asarray(Wo, np.float32)[cols, :]
            .reshape(DG // P, P, D)
            .transpose(1, 0, 2)
            .astype(BF16NP)
        )
        in_maps.append(
            {
                "xT": np.ascontiguousarray(x[b].T.astype(BF16NP)),
                "wq": wslice(Wq),
                "wk": wslice(Wk),
                "wv": wslice(Wv),
                "wo": wo_c,
            }
        )
    return in_maps


def kernel(x, Wq, Wk, Wv, Wo, bo):
    nc = build_nc()
    in_maps = make_in_maps(x, Wq, Wk, Wv, Wo)
    trace = bool(int(os.environ.get("MHA_TRACE", "0")))
    if trace:
        _install_ntff_hook()
    res = run_bass_kernel_spmd(
        nc, in_maps, core_ids=list(range(NCORES)), trace=trace,
        trace_cores=list(range(NCORES)) if trace else None,
    )
    _CACHE["last_results"] = res
    bo = np.asarray(bo, np.float32)
    out = np.zeros((B, S, D), np.float32)
    for c in range(NCORES):
        out[c // GROUPS] += np.asarray(res.results[c]["out"], dtype=np.float32)
    out += bo[None, None, :]
    return out


# revision 18
# speedup vs baseline: 1.2924x; 1.0106x over previous
"""Multi-head causal attention (B=2, S=2048, D=1024, H=16) on 8 trn2 cores.

Sharding: core c handles batch b = c // 4 and head group g = c % 4 (4 heads,
256 feature columns). Each core computes its heads' attention context and a
partial output projection (ctx_g @ Wo[rows_g]); the host sums the 4 partials
per batch (upcasting the bf16 partials) and adds bo.

v4 layout (all matmul operands bf16, fp32 PSUM accumulate):
- x is host-transposed to xT [D, S] bf16 so the QKV contraction dim sits on
  SBUF partitions; Q^T/K^T are produced head-major (head h on partitions
  [64*(h%2):+64] of tile m=h//2) so score matmuls contract 64 partitions with
  matching base partitions and no transposes; P^T = exp(S^T) feeds the PV
  matmul directly as the moving operand.
- Attention is sq-half-major (sq halves of 1024): per (head, sk-tile) one
  [128,1024] score psum (matmuls segmented at 512 -- PSUM bank limit), ONE
  wide exp (bf16 out), gpsimd affine_select on the diagonal block, PV
  accumulated into per-(head,half,512-seg) ctx psums with exact chain flags.
- Softmax denominator: ones column folded into V; normalize at the ctx
  psum->sbuf copy via DVE row copy + gpsimd partition_broadcast +
  reciprocal_approx_fast (no DRAM round trip, no slow full reciprocal);
  two-phase emission keeps the broadcast from stalling the gpsimd queue.
- PSUM: scores 2x[128,1024] (4 banks) + ctx 3x[128,512] (3) + filler
  1x[128,512] (1). The filler bank runs projection / out-projection chains
  dripped one per attention step, so the PE always has work while the scalar
  engine streams the ~80us of exps -- keeping the PE p-state at full clock.
"""

import os
import sys
import types
from contextlib import ExitStack

import numpy as np
import ml_dtypes

import concourse.bacc as bacc
import concourse.bass as bass
import concourse.mybir as mybir
import concourse.tile as tile
from concourse.bass_utils import run_bass_kernel_spmd


def _install_ntff_hook():
    """The agent image's antenv lacks axon_hooks, so trn_boot's NTFF hook
    install degrades silently. Recreate the module + hook so trace=True works."""
    if "antenv.axon_hooks" in sys.modules:
        return
    try:
        mod = types.ModuleType("antenv.axon_hooks")
        holder = [None]
        mod.set_axon_ntff_profile_hook = lambda h: holder.__setitem__(0, h)
        mod.get_axon_ntff_profile_hook = lambda: holder[0]
        from trn_agent_boot.trn_boot import _ntff_profile_via_ctypes

        hook = _ntff_profile_via_ctypes("/opt/axon/libaxon_pjrt.so")
        if hook is None:
            return
        mod.set_axon_ntff_profile_hook(hook)
        sys.modules["antenv.axon_hooks"] = mod
    except Exception:
        pass


B, S, D, H, HD = 2, 2048, 1024, 16, 64
NCORES = 8
GROUPS = 4          # head groups (cores) per batch
HC = H // GROUPS    # heads per core
DG = HC * HD        # feature columns per core (256)
P = 128
KSUB = D // P       # 8 contraction subtiles for the projections
SLC = 1024          # projection s-slice / attention half width
NST = S // P        # 16 sk subtiles of 128
F32 = mybir.dt.float32
BF16 = mybir.dt.bfloat16
BF16NP = ml_dtypes.bfloat16

_CACHE = {}


class _MHA:
    """Holds tile handles so emission helpers can be interleaved freely."""

    def __init__(self, tc, ctx, xT, wq, wk, wv, wo, out):
        self.tc = tc
        self.nc = tc.nc
        self.out = out
        self.scale = 1.0 / float(np.sqrt(np.float32(HD)))
        nc = self.nc

        self.consts = ctx.enter_context(tc.tile_pool(name="consts", bufs=1))
        self.sps = ctx.enter_context(tc.tile_pool(name="sps", bufs=2, space="PSUM"))
        self.cps = ctx.enter_context(tc.tile_pool(name="cps", bufs=3, space="PSUM"))
        self.fps = ctx.enter_context(tc.tile_pool(name="fps", bufs=1, space="PSUM"))
        self.xw = ctx.enter_context(tc.tile_pool(name="xw", bufs=2))
        self.ptp = ctx.enter_context(tc.tile_pool(name="ptp", bufs=4))
        self.smalls = ctx.enter_context(tc.tile_pool(name="smalls", bufs=4))
        self.bcp = ctx.enter_context(tc.tile_pool(name="bcp", bufs=3))
        self.outp = ctx.enter_context(tc.tile_pool(name="outp", bufs=3))

        c = self.consts
        self.qt = c.tile([P, 2, S], BF16)    # head h rows at [64*(h%2), h//2]
        self.kt = c.tile([P, 2, S], BF16)
        self.vt = c.tile([P, NST, HC, P], BF16)  # [sk, sst, h, 64v+1+63pad]
        self.ctxt = c.tile([P, 2, S], BF16)  # normalized ctx^T, qt layout
        self.wq_sb = c.tile([P, KSUB, DG], BF16)
        self.wk_sb = c.tile([P, KSUB, DG], BF16)
        self.wv_sb = c.tile([P, KSUB, DG], BF16)
        self.wo_sb = c.tile([P, DG // P, D], BF16)
        nc.sync.dma_start(out=self.wq_sb, in_=wq)
        nc.sync.dma_start(out=self.wk_sb, in_=wk)
        nc.sync.dma_start(out=self.wv_sb, in_=wv)
        nc.sync.dma_start(out=self.wo_sb, in_=wo)

        # V pad columns must be zero (psum garbage must stay finite); one-off
        # on gpsimd, off the DVE critical path.
        nc.gpsimd.memset(self.vt, 0.0)
        osc = c.tile([P, 1], F32)
        nc.vector.memset(osc, 1.0)
        for h in range(HC):
            ones_col = 64 if h % 2 == 0 else 0
            nc.vector.tensor_copy(
                out=self.vt[:, :, h, ones_col : ones_col + 1],
                in_=osc[:, None, :].to_broadcast((P, NST, 1)),
            )

        self.xT = xT
        self.xn = [None, None]
        self.cpsum = {}  # (h, half, seg) -> [128,512] psum tile
        self.pending_norm = []

    # ---- projections (filler units on the 1-bank fps pool) ------------
    def dma_x(self, n):
        xn = self.xw.tile([P, KSUB, SLC], BF16, tag="xn", name=f"xn_{n}")
        for k in range(KSUB):
            self.nc.sync.dma_start(
                out=xn[:, k, :], in_=self.xT[k * P : (k + 1) * P, n * SLC : (n + 1) * SLC]
            )
        self.xn[n] = xn

    def proj_qk(self, n, m, which, half):
        """One [128,512] chain of Q^T or K^T: slice n, head-pair m, 512-half."""
        nc = self.nc
        w = self.wq_sb if which == "q" else self.wk_sb
        dst = self.qt if which == "q" else self.kt
        ps = self.fps.tile([P, 512], F32, tag="f", name=f"{which}ps_{n}_{m}_{half}")
        for k in range(KSUB):
            nc.tensor.matmul(
                ps,
                lhsT=w[:, k, m * P : (m + 1) * P],
                rhs=self.xn[n][:, k, half * 512 : (half + 1) * 512],
                start=(k == 0),
                stop=(k == KSUB - 1),
            )
        c0 = n * SLC + half * 512
        nc.vector.tensor_copy(out=dst[:, m, c0 : c0 + 512], in_=ps)

    def proj_v(self, n, j2):
        """V for s-subtiles (8n+2*j2, 8n+2*j2+1) -> vt rows with ones layout."""
        nc = self.nc
        ps = self.fps.tile([P, 512], F32, tag="f", name=f"vps_{n}_{j2}")
        for j in range(2):
            for k in range(KSUB):
                nc.tensor.matmul(
                    ps[:, j * DG : (j + 1) * DG],
                    lhsT=self.xn[n][:, k, (2 * j2 + j) * P : (2 * j2 + j + 1) * P],
                    rhs=self.wv_sb[:, k, :],
                    start=(k == 0),
                    stop=(k == KSUB - 1),
                )
        st0 = 8 * n + 2 * j2
        psv = ps.rearrange("p (t h d) -> p t h d", h=HC, d=HD)
        nc.vector.tensor_copy(
            out=self.vt[:, st0 : st0 + 2, 0:HC:2, 0:HD], in_=psv[:, :, 0:HC:2, :]
        )
        nc.vector.tensor_copy(
            out=self.vt[:, st0 : st0 + 2, 1:HC:2, HD:P], in_=psv[:, :, 1:HC:2, :]
        )

    # ---- attention ----------------------------------------------------
    def att_step(self, h, half, ski):
        """One (head, sk-tile) step in an sq half: segmented score matmuls,
        one wide exp, diagonal mask, segmented PV accumulate."""
        nc = self.nc
        hp = 64 * (h % 2)
        hm = h // 2
        g0 = half * SLC            # global sq start of this half
        a = max(g0, P * ski)       # global sq start of the valid window
        w = g0 + SLC - a
        if w <= 0:
            return
        diag = P * ski >= g0
        l0 = a - g0
        segs = [s for s in ((l0, min(512, SLC)), (max(l0, 512), SLC)) if s[0] < s[1]]

        spsum = self.sps.tile([P, SLC], F32, tag="ps", name=f"s_{h}_{half}_{ski}")
        for s0, s1 in segs:
            nc.tensor.matmul(
                spsum[:, s0:s1],
                lhsT=self.kt[hp : hp + 64, hm, ski * P : (ski + 1) * P],
                rhs=self.qt[hp : hp + 64, hm, g0 + s0 : g0 + s1],
                start=True,
                stop=True,
            )
        pt = self.ptp.tile([P, SLC], BF16, tag="pt", name=f"pt_{h}_{half}_{ski}")
        nc.scalar.activation(
            out=pt[:, 0:w],
            in_=spsum[:, l0:SLC],
            func=mybir.ActivationFunctionType.Exp,
            bias=0.0,
            scale=float(self.scale),
        )
        if diag:
            nc.gpsimd.affine_select(
                out=pt[:, 0:P],
                in_=pt[:, 0:P],
                pattern=[[1, P]],
                compare_op=mybir.AluOpType.is_ge,
                fill=0.0,
                base=0,
                channel_multiplier=-1,
            )
        for s0, s1 in segs:
            seg = s0 >= 512
            sqt = 2 * half + seg
            key = (h, half, seg)
            if key not in self.cpsum:
                self.cpsum[key] = self.cps.tile(
                    [P, 512], F32, tag="ctx", name=f"ctx_{h}_{half}_{seg}"
                )
            nc.tensor.matmul(
                self.cpsum[key][:, s0 - 512 * seg : s1 - 512 * seg],
                lhsT=self.vt[:, ski, h, :],
                rhs=pt[:, s0 - l0 : s1 - l0],
                start=(ski == 0),
                stop=(ski == 4 * sqt + 3),
            )
        if ski % 4 == 3:
            sqt = ski // 4
            if sqt in (2 * half, 2 * half + 1):
                self.norm_start(h, half, sqt)

    def norm_start(self, h, half, sqt):
        """Phase 1 of the eager normalize: pull the denominator row out of
        the finished ctx psum. Phase 2 is flushed later (one sq-tile lag)."""
        nc = self.nc
        denom_row = 64 if h % 2 == 0 else 0
        cpsum = self.cpsum[(h, half, sqt % 2)]
        den = self.smalls.tile([1, 512], F32, tag="den", name=f"den_{h}_{sqt}")
        nc.vector.tensor_copy(out=den, in_=cpsum[denom_row : denom_row + 1, :])
        self.pending_norm.append((h, half, sqt, den))

    def norm_flush(self):
        nc = self.nc
        for h, half, sqt, den in self.pending_norm:
            hp = 64 * (h % 2)
            hm = h // 2
            ctx_rows = 0 if h % 2 == 0 else 64
            cpsum = self.cpsum[(h, half, sqt % 2)]
            sq0 = sqt * 512
            bc = self.bcp.tile([P, 512], F32, tag="bc", name=f"bc_{h}_{sqt}")
            nc.gpsimd.partition_broadcast(bc, den, channels=P)
            nc.vector.reciprocal_approx_fast(out=bc, in_=bc)
            nc.vector.tensor_tensor(
                self.ctxt[hp : hp + 64, hm, sq0 : sq0 + 512],
                cpsum[ctx_rows : ctx_rows + 64, :],
                bc[ctx_rows : ctx_rows + 64, :],
                mybir.AluOpType.mult,
            )
        self.pending_norm = []

    # ---- output projection (filler units) -----------------------------
    def outproj(self, st, nn):
        nc = self.nc
        ps = self.fps.tile([P, 512], F32, tag="f", name=f"ops_{st}_{nn}")
        for k in range(DG // P):
            nc.tensor.matmul(
                ps,
                lhsT=self.ctxt[:, k, st * P : (st + 1) * P],
                rhs=self.wo_sb[:, k, nn * 512 : (nn + 1) * 512],
                start=(k == 0),
                stop=(k == DG // P - 1),
            )
        ot = self.outp.tile([P, 512], BF16, tag="ot", name=f"ot_{st}_{nn}")
        nc.vector.tensor_copy(out=ot, in_=ps)
        nc.sync.dma_start(
            out=self.out[st * P : (st + 1) * P, nn * 512 : (nn + 1) * 512], in_=ot
        )


def _emit(tc, xT, wq, wk, wv, wo, out):
    with ExitStack() as ctx:
        m = _MHA(tc, ctx, xT, wq, wk, wv, wo, out)

        # Minimal lead-in: x slice 0, then only the Q/K tiles head 0 needs.
        m.dma_x(0)
        m.proj_qk(0, 0, "q", 0)
        m.proj_qk(0, 0, "q", 1)
        m.proj_qk(0, 0, "k", 0)
        m.proj_qk(0, 0, "k", 1)
        m.dma_x(1)

        # Remaining projection work becomes filler units, dripped one per
        # attention step so the PE never idles while ACT streams the exps.
        # V units for slice 0 MUST be emitted before the h0 att_step that
        # first consumes them (PV of ski needs vt s-subtile ski).
        fill = []
        fill += [lambda hf=hf: m.proj_qk(0, 1, "q", hf) for hf in range(2)]
        fill += [lambda hf=hf: m.proj_qk(0, 1, "k", hf) for hf in range(2)]
        fill += [lambda hf=hf: m.proj_qk(1, 0, "q", hf) for hf in range(2)]
        fill += [lambda hf=hf: m.proj_qk(1, 0, "k", hf) for hf in range(2)]
        fill += [lambda j2=j2: m.proj_v(1, j2) for j2 in range(4)]
        fill += [lambda hf=hf: m.proj_qk(1, 1, "q", hf) for hf in range(2)]
        fill += [lambda hf=hf: m.proj_qk(1, 1, "k", hf) for hf in range(2)]
        fi = 0

        # half 0: heads 0..3 over ski 0..7
        for h in range(HC):
            for ski in range(8):
                if h == 0 and ski % 2 == 0:
                    m.proj_v(0, ski // 2)   # covers s-subtiles ski, ski+1
                m.att_step(h, 0, ski)
                if h >= 1 and fi < len(fill):
                    fill[fi](); fi += 1
                if ski % 4 == 1:
                    m.norm_flush()
            m.norm_flush()
        while fi < len(fill):
            fill[fi](); fi += 1

        # half 1: heads 0..3 over ski 0..15; out-projection of the finished
        # first half (st 0..7) is the filler stream, spread thinly so the PE
        # never idles long enough to drop its p-state. st 8..11 unlock once
        # h3's sq-tile-2 normalize is flushed (h3 ski 13).
        ops = [(st, nn) for st in range(8) for nn in range(2)]
        oi = 0
        step = 0
        for h in range(HC):
            for ski in range(16):
                m.att_step(h, 1, ski)
                step += 1
                if step % 3 == 0 and oi < len(ops):
                    st, nn = ops[oi]; m.outproj(st, nn); oi += 1
                if ski % 4 == 1:
                    m.norm_flush()
                if h == 3 and ski >= 14:
                    for st in (2 * ski - 20, 2 * ski - 19):  # st 8,9 @14; 10,11 @15
                        m.outproj(st, 0)
                        m.outproj(st, 1)
            m.norm_flush()
        while oi < len(ops):
            st, nn = ops[oi]; m.outproj(st, nn); oi += 1

        # tail: st 12..15 need h3's last normalize
        for st in range(12, NST):
            for nn in range(2):
                m.outproj(st, nn)


def build_nc():
    if "nc" in _CACHE:
        return _CACHE["nc"]
    nc = bacc.Bacc("TRN2", target_bir_lowering=False, debug=False, num_devices=NCORES)
    xT = nc.dram_tensor("xT", (D, S), BF16, kind="ExternalInput").ap()
    wq = nc.dram_tensor("wq", (P, KSUB, DG), BF16, kind="ExternalInput").ap()
    wk = nc.dram_tensor("wk", (P, KSUB, DG), BF16, kind="ExternalInput").ap()
    wv = nc.dram_tensor("wv", (P, KSUB, DG), BF16, kind="ExternalInput").ap()
    wo = nc.dram_tensor("wo", (P, DG // P, D), BF16, kind="ExternalInput").ap()
    out = nc.dram_tensor("out", (S, D), BF16, kind="ExternalOutput").ap()
    with tile.TileContext(nc) as tc:
        _emit(tc, xT, wq, wk, wv, wo, out)
    nc.compile()
    _CACHE["nc"] = nc
    return nc


def make_in_maps(x, Wq, Wk, Wv, Wo):
    x = np.asarray(x, np.float32)
    in_maps = []
    for c in range(NCORES):
        b, g = c // GROUPS, c % GROUPS
        cols = slice(g * DG, (g + 1) * DG)

        def wslice(W):
            # [D, DG] -> [128, KSUB, DG] with [p, k, m] = W[k*128+p, m]
            return np.ascontiguousarray(
                np.asarray(W, np.float32)[:, cols]
                .reshape(KSUB, P, DG)
                .transpose(1, 0, 2)
                .astype(BF16NP)
            )

        wo_c = np.ascontiguousarray(
            np.asarray(Wo, np.float32)[cols, :]
            .reshape(DG // P, P, D)
            .transpose(1, 0, 2)
            .astype(BF16NP)
        )
        in_maps.append(
            {
                "xT": np.ascontiguousarray(x[b].T.astype(BF16NP)),
                "wq": wslice(Wq),
                "wk": wslice(Wk),
                "wv": wslice(Wv),
                "wo": wo_c,
            }
        )
    return in_maps


def kernel(x, Wq, Wk, Wv, Wo, bo):
    nc = build_nc()
    in_maps = make_in_maps(x, Wq, Wk, Wv, Wo)
    trace = bool(int(os.environ.get("MHA_TRACE", "0")))
    if trace:
        _install_ntff_hook()
    res = run_bass_kernel_spmd(
        nc, in_maps, core_ids=list(range(NCORES)), trace=trace,
        trace_cores=list(range(NCORES)) if trace else None,
    )
    _CACHE["last_results"] = res
    bo = np.asarray(bo, np.float32)
    out = np.zeros((B, S, D), np.float32)
    for c in range(NCORES):
        out[c // GROUPS] += np.asarray(res.results[c]["out"], dtype=np.float32)
    out += bo[None, None, :]
    return out


# revision 25
# speedup vs baseline: 1.3498x; 1.0444x over previous
"""Multi-head causal attention (B=2, S=2048, D=1024, H=16) on 8 trn2 cores.

Sharding: core c handles batch b = c // 4 and head group g = c % 4 (4 heads,
256 feature columns). Each core computes its heads' attention context and a
partial output projection (ctx_g @ Wo[rows_g]); the host sums the 4 partials
per batch (upcasting the bf16 partials) and adds bo.

v4 layout (all matmul operands bf16, fp32 PSUM accumulate):
- x is host-transposed to xT [D, S] bf16 so the QKV contraction dim sits on
  SBUF partitions; Q^T/K^T are produced head-major (head h on partitions
  [64*(h%2):+64] of tile m=h//2) so score matmuls contract 64 partitions with
  matching base partitions and no transposes; P^T = exp(S^T) feeds the PV
  matmul directly as the moving operand.
- Attention is sq-half-major (sq halves of 1024): per (head, sk-tile) one
  [128,1024] score psum (matmuls segmented at 512 -- PSUM bank limit), ONE
  wide exp (bf16 out), gpsimd affine_select on the diagonal block, PV
  accumulated into per-(head,half,512-seg) ctx psums with exact chain flags.
- Softmax denominator: ones column folded into V; normalize at the ctx
  psum->sbuf copy via DVE row copy + gpsimd partition_broadcast +
  reciprocal_approx_fast (no DRAM round trip, no slow full reciprocal);
  two-phase emission keeps the broadcast from stalling the gpsimd queue.
- PSUM: scores 2x[128,1024] (4 banks) + ctx 3x[128,512] (3) + filler
  1x[128,512] (1). The filler bank runs projection / out-projection chains
  dripped one per attention step, so the PE always has work while the scalar
  engine streams the ~80us of exps -- keeping the PE p-state at full clock.
"""

import os
import sys
import types
from contextlib import ExitStack

import numpy as np
import ml_dtypes

import concourse.bacc as bacc
import concourse.bass as bass
import concourse.mybir as mybir
import concourse.tile as tile
from concourse.bass_utils import run_bass_kernel_spmd


def _install_ntff_hook():
    """The agent image's antenv lacks axon_hooks, so trn_boot's NTFF hook
    install degrades silently. Recreate the module + hook so trace=True works."""
    if "antenv.axon_hooks" in sys.modules:
        return
    try:
        mod = types.ModuleType("antenv.axon_hooks")
        holder = [None]
        mod.set_axon_ntff_profile_hook = lambda h: holder.__setitem__(0, h)
        mod.get_axon_ntff_profile_hook = lambda: holder[0]
        from trn_agent_boot.trn_boot import _ntff_profile_via_ctypes

        hook = _ntff_profile_via_ctypes("/opt/axon/libaxon_pjrt.so")
        if hook is None:
            return
        mod.set_axon_ntff_profile_hook(hook)
        sys.modules["antenv.axon_hooks"] = mod
    except Exception:
        pass


B, S, D, H, HD = 2, 2048, 1024, 16, 64
NCORES = 8
GROUPS = 4          # head groups (cores) per batch
HC = H // GROUPS    # heads per core
DG = HC * HD        # feature columns per core (256)
P = 128
KSUB = D // P       # 8 contraction subtiles for the projections
SLC = 1024          # projection s-slice / attention half width
NST = S // P        # 16 sk subtiles of 128
F32 = mybir.dt.float32
BF16 = mybir.dt.bfloat16
BF16NP = ml_dtypes.bfloat16

_CACHE = {}


class _MHA:
    """Holds tile handles so emission helpers can be interleaved freely."""

    def __init__(self, tc, ctx, xT, wq, wk, wv, wo, out):
        self.tc = tc
        self.nc = tc.nc
        self.out = out
        self.scale = 1.0 / float(np.sqrt(np.float32(HD)))
        nc = self.nc

        self.consts = ctx.enter_context(tc.tile_pool(name="consts", bufs=1))
        self.sps = ctx.enter_context(tc.tile_pool(name="sps", bufs=2, space="PSUM"))
        self.cps = ctx.enter_context(tc.tile_pool(name="cps", bufs=3, space="PSUM"))
        self.fps = ctx.enter_context(tc.tile_pool(name="fps", bufs=1, space="PSUM"))
        self.xw = ctx.enter_context(tc.tile_pool(name="xw", bufs=2))
        self.ptp = ctx.enter_context(tc.tile_pool(name="ptp", bufs=4))
        self.smalls = ctx.enter_context(tc.tile_pool(name="smalls", bufs=4))
        self.bcp = ctx.enter_context(tc.tile_pool(name="bcp", bufs=3))
        self.outp = ctx.enter_context(tc.tile_pool(name="outp", bufs=3))

        c = self.consts
        self.qt = c.tile([P, 2, S], BF16)    # head h rows at [64*(h%2), h//2]
        self.kt = c.tile([P, 2, S], BF16)
        self.vt = c.tile([P, NST, HC, P], BF16)  # [sk, sst, h, 64v+1+63pad]
        self.ctxt = c.tile([P, 2, S], BF16)  # normalized ctx^T, qt layout
        self.wq_sb = c.tile([P, KSUB, DG], BF16)
        self.wk_sb = c.tile([P, KSUB, DG], BF16)
        self.wv_sb = c.tile([P, KSUB, DG], BF16)
        self.wo_sb = c.tile([P, DG // P, D], BF16)
        # issued in consumption order (wq before x before wk ...; wo last)
        nc.sync.dma_start(out=self.wq_sb, in_=wq)
        self._w_dmas = (wk, wv, wo)

        # V pad columns must be zero (psum garbage must stay finite); one-off
        # on gpsimd, off the DVE critical path.
        nc.gpsimd.memset(self.vt, 0.0)
        osc = c.tile([P, 1], F32)
        nc.vector.memset(osc, 1.0)
        for h in range(HC):
            ones_col = 64 if h % 2 == 0 else 0
            nc.vector.tensor_copy(
                out=self.vt[:, :, h, ones_col : ones_col + 1],
                in_=osc[:, None, :].to_broadcast((P, NST, 1)),
            )

        self.xT = xT
        self.xn = [None, None]
        self.cpsum = {}  # (h, half, seg) -> [128,512] psum tile
        self.pending_norm = []

    # ---- projections (filler units on the 1-bank fps pool) ------------
    def dma_x(self, n):
        xn = self.xw.tile([P, KSUB, SLC], BF16, tag="xn", name=f"xn_{n}")
        for k in range(KSUB):
            self.nc.sync.dma_start(
                out=xn[:, k, :], in_=self.xT[k * P : (k + 1) * P, n * SLC : (n + 1) * SLC]
            )
        self.xn[n] = xn

    def proj_qk(self, n, m, which, half):
        """One [128,512] chain of Q^T or K^T: slice n, head-pair m, 512-half."""
        nc = self.nc
        w = self.wq_sb if which == "q" else self.wk_sb
        dst = self.qt if which == "q" else self.kt
        ps = self.fps.tile([P, 512], F32, tag="f", name=f"{which}ps_{n}_{m}_{half}")
        for k in range(KSUB):
            nc.tensor.matmul(
                ps,
                lhsT=w[:, k, m * P : (m + 1) * P],
                rhs=self.xn[n][:, k, half * 512 : (half + 1) * 512],
                start=(k == 0),
                stop=(k == KSUB - 1),
            )
        c0 = n * SLC + half * 512
        nc.vector.tensor_copy(out=dst[:, m, c0 : c0 + 512], in_=ps)

    def proj_v(self, n, j2):
        """V for s-subtiles (8n+2*j2, 8n+2*j2+1) -> vt rows with ones layout."""
        nc = self.nc
        ps = self.fps.tile([P, 512], F32, tag="f", name=f"vps_{n}_{j2}")
        for j in range(2):
            for k in range(KSUB):
                nc.tensor.matmul(
                    ps[:, j * DG : (j + 1) * DG],
                    lhsT=self.xn[n][:, k, (2 * j2 + j) * P : (2 * j2 + j + 1) * P],
                    rhs=self.wv_sb[:, k, :],
                    start=(k == 0),
                    stop=(k == KSUB - 1),
                )
        st0 = 8 * n + 2 * j2
        psv = ps.rearrange("p (t h d) -> p t h d", h=HC, d=HD)
        nc.vector.tensor_copy(
            out=self.vt[:, st0 : st0 + 2, 0:HC:2, 0:HD], in_=psv[:, :, 0:HC:2, :]
        )
        nc.vector.tensor_copy(
            out=self.vt[:, st0 : st0 + 2, 1:HC:2, HD:P], in_=psv[:, :, 1:HC:2, :]
        )

    # ---- attention ----------------------------------------------------
    def att_step(self, h, half, ski, fill_cb=None):
        """One (head, sk-tile) step in an sq half: segmented score matmuls,
        one wide exp, diagonal mask, segmented PV accumulate. fill_cb (if
        given) is emitted between the exp and the PV matmuls, so filler PE
        work lands where the PE would otherwise wait on the exp."""
        nc = self.nc
        hp = 64 * (h % 2)
        hm = h // 2
        g0 = half * SLC            # global sq start of this half
        a = max(g0, P * ski)       # global sq start of the valid window
        w = g0 + SLC - a
        if w <= 0:
            return
        diag = P * ski >= g0
        l0 = a - g0
        segs = [s for s in ((l0, min(512, SLC)), (max(l0, 512), SLC)) if s[0] < s[1]]

        spsum = self.sps.tile([P, SLC], F32, tag="ps", name=f"s_{h}_{half}_{ski}")
        for s0, s1 in segs:
            nc.tensor.matmul(
                spsum[:, s0:s1],
                lhsT=self.kt[hp : hp + 64, hm, ski * P : (ski + 1) * P],
                rhs=self.qt[hp : hp + 64, hm, g0 + s0 : g0 + s1],
                start=True,
                stop=True,
            )
        pt = self.ptp.tile([P, SLC], BF16, tag="pt", name=f"pt_{h}_{half}_{ski}")
        nc.scalar.activation(
            out=pt[:, 0:w],
            in_=spsum[:, l0:SLC],
            func=mybir.ActivationFunctionType.Exp,
            bias=0.0,
            scale=float(self.scale),
        )
        if diag:
            nc.gpsimd.affine_select(
                out=pt[:, 0:P],
                in_=pt[:, 0:P],
                pattern=[[1, P]],
                compare_op=mybir.AluOpType.is_ge,
                fill=0.0,
                base=0,
                channel_multiplier=-1,
            )
        if fill_cb is not None:
            fill_cb()
        for s0, s1 in segs:
            seg = s0 >= 512
            sqt = 2 * half + seg
            key = (h, half, seg)
            if key not in self.cpsum:
                self.cpsum[key] = self.cps.tile(
                    [P, 512], F32, tag="ctx", name=f"ctx_{h}_{half}_{seg}"
                )
            nc.tensor.matmul(
                self.cpsum[key][:, s0 - 512 * seg : s1 - 512 * seg],
                lhsT=self.vt[:, ski, h, :],
                rhs=pt[:, s0 - l0 : s1 - l0],
                start=(ski == 0),
                stop=(ski == 4 * sqt + 3),
            )
        if ski % 4 == 3:
            sqt = ski // 4
            if sqt in (2 * half, 2 * half + 1):
                self.norm_start(h, half, sqt)

    def norm_start(self, h, half, sqt):
        """Phase 1 of the eager normalize: pull the denominator row out of
        the finished ctx psum. Phase 2 is flushed later (one sq-tile lag)."""
        nc = self.nc
        denom_row = 64 if h % 2 == 0 else 0
        cpsum = self.cpsum[(h, half, sqt % 2)]
        den = self.smalls.tile([1, 512], F32, tag="den", name=f"den_{h}_{sqt}")
        nc.vector.tensor_copy(out=den, in_=cpsum[denom_row : denom_row + 1, :])
        self.pending_norm.append((h, half, sqt, den))

    def norm_flush(self):
        nc = self.nc
        for h, half, sqt, den in self.pending_norm:
            hp = 64 * (h % 2)
            hm = h // 2
            ctx_rows = 0 if h % 2 == 0 else 64
            cpsum = self.cpsum[(h, half, sqt % 2)]
            sq0 = sqt * 512
            bc = self.bcp.tile([P, 512], F32, tag="bc", name=f"bc_{h}_{sqt}")
            nc.gpsimd.partition_broadcast(bc, den, channels=P)
            nc.vector.reciprocal_approx_fast(out=bc, in_=bc)
            nc.vector.tensor_tensor(
                self.ctxt[hp : hp + 64, hm, sq0 : sq0 + 512],
                cpsum[ctx_rows : ctx_rows + 64, :],
                bc[ctx_rows : ctx_rows + 64, :],
                mybir.AluOpType.mult,
            )
        self.pending_norm = []

    # ---- output projection (filler units) -----------------------------
    def outproj(self, st, nn, pool=None):
        """pool='ctx' uses the (freed) ctx psum rotation -- for the tail,
        where the 3-deep rotation hides the psum->sbuf copies; the default
        1-bank filler pool serializes unit-to-unit."""
        nc = self.nc
        if pool == "ctx":
            ps = self.cps.tile([P, 512], F32, tag="ctx", name=f"ops_{st}_{nn}")
        else:
            ps = self.fps.tile([P, 512], F32, tag="f", name=f"ops_{st}_{nn}")
        for k in range(DG // P):
            nc.tensor.matmul(
                ps,
                lhsT=self.ctxt[:, k, st * P : (st + 1) * P],
                rhs=self.wo_sb[:, k, nn * 512 : (nn + 1) * 512],
                start=(k == 0),
                stop=(k == DG // P - 1),
            )
        ot = self.outp.tile([P, 512], BF16, tag="ot", name=f"ot_{st}_{nn}")
        nc.vector.tensor_copy(out=ot, in_=ps)
        nc.sync.dma_start(
            out=self.out[st * P : (st + 1) * P, nn * 512 : (nn + 1) * 512], in_=ot
        )


def _emit(tc, xT, wq, wk, wv, wo, out):
    with ExitStack() as ctx:
        m = _MHA(tc, ctx, xT, wq, wk, wv, wo, out)

        # Minimal lead-in: wq is already in flight; x slice 0, then the
        # other weights behind it, then only the Q/K tiles head 0 needs.
        m.dma_x(0)
        wk_d, wv_d, wo_d = m._w_dmas
        m.nc.sync.dma_start(out=m.wk_sb, in_=wk_d)
        m.dma_x(1)
        m.nc.sync.dma_start(out=m.wv_sb, in_=wv_d)
        m.nc.sync.dma_start(out=m.wo_sb, in_=wo_d)
        m.proj_qk(0, 0, "q", 0)
        m.proj_qk(0, 0, "k", 0)
        m.proj_qk(0, 0, "q", 1)
        m.proj_qk(0, 0, "k", 1)

        # Remaining projection work becomes filler units, dripped one per
        # attention step so the PE never idles while ACT streams the exps.
        # V units for slice 0 MUST be emitted before the h0 att_step that
        # first consumes them (PV of ski needs vt s-subtile ski).
        fill = []
        fill += [lambda hf=hf: m.proj_qk(0, 1, "q", hf) for hf in range(2)]
        fill += [lambda hf=hf: m.proj_qk(0, 1, "k", hf) for hf in range(2)]
        fill += [lambda hf=hf: m.proj_qk(1, 0, "q", hf) for hf in range(2)]
        fill += [lambda hf=hf: m.proj_qk(1, 0, "k", hf) for hf in range(2)]
        fill += [lambda j2=j2: m.proj_v(1, j2) for j2 in range(4)]
        fill += [lambda hf=hf: m.proj_qk(1, 1, "q", hf) for hf in range(2)]
        fill += [lambda hf=hf: m.proj_qk(1, 1, "k", hf) for hf in range(2)]
        fi = 0

        # half 0: heads 0..3 over ski 0..7. h0's V units are emitted inside
        # the att_step (between exp and PV) so the first exps aren't delayed,
        # while still preceding the PV matmuls that consume them.
        for h in range(HC):
            for ski in range(8):
                cb = None
                if h == 0 and ski % 2 == 0:
                    cb = (lambda j2=ski // 2: m.proj_v(0, j2))
                m.att_step(h, 0, ski, fill_cb=cb)
                if h >= 1 and fi < len(fill):
                    fill[fi](); fi += 1
                if ski % 4 == 1:
                    m.norm_flush()
            m.norm_flush()
        while fi < len(fill):
            fill[fi](); fi += 1

        # half 1: heads 0..3 over ski 0..15; out-projection of the finished
        # first half (st 0..7) is the filler stream, spread thinly so the PE
        # never idles long enough to drop its p-state. st 8..11 unlock once
        # h3's sq-tile-2 normalize is flushed (h3 ski 13).
        ops = [(st, nn) for st in range(8) for nn in range(2)]
        oi = 0
        step = 0
        for h in range(HC):
            for ski in range(16):
                m.att_step(h, 1, ski)
                step += 1
                if step % 3 == 0 and oi < len(ops):
                    st, nn = ops[oi]; m.outproj(st, nn); oi += 1
                if ski % 4 == 1:
                    m.norm_flush()
                if h == 3 and ski == 14:
                    # h3's sq-tile-2 normalize flushed at ski 13: st8 can go
                    # on the two free ctx psum slots
                    m.outproj(8, 0, pool="ctx")
                    m.outproj(8, 1, pool="ctx")
            m.norm_flush()
        while oi < len(ops):
            st, nn = ops[oi]; m.outproj(st, nn); oi += 1

        # tail on the freed ctx psum rotation (hides the psum->sbuf copies)
        for st in range(9, NST):
            for nn in range(2):
                m.outproj(st, nn, pool="ctx")


def build_nc():
    if "nc" in _CACHE:
        return _CACHE["nc"]
    nc = bacc.Bacc("TRN2", target_bir_lowering=False, debug=False, num_devices=NCORES)
    xT = nc.dram_tensor("xT", (D, S), BF16, kind="ExternalInput").ap()
    wq = nc.dram_tensor("wq", (P, KSUB, DG), BF16, kind="ExternalInput").ap()
    wk = nc.dram_tensor("wk", (P, KSUB, DG), BF16, kind="ExternalInput").ap()
    wv = nc.dram_tensor("wv", (P, KSUB, DG), BF16, kind="ExternalInput").ap()
    wo = nc.dram_tensor("wo", (P, DG // P, D), BF16, kind="ExternalInput").ap()
    out = nc.dram_tensor("out", (S, D), BF16, kind="ExternalOutput").ap()
    with tile.TileContext(nc) as tc:
        _emit(tc, xT, wq, wk, wv, wo, out)
    nc.compile()
    _CACHE["nc"] = nc
    return nc


def make_in_maps(x, Wq, Wk, Wv, Wo):
    x = np.asarray(x, np.float32)
    in_maps = []
    for c in range(NCORES):
        b, g = c // GROUPS, c % GROUPS
        cols = slice(g * DG, (g + 1) * DG)

        def wslice(W):
            # [D, DG] -> [128, KSUB, DG] with [p, k, m] = W[k*128+p, m]
            return np.ascontiguousarray(
                np.asarray(W, np.float32)[:, cols]
                .reshape(KSUB, P, DG)
                .transpose(1, 0, 2)
                .astype(BF16NP)
            )

        wo_c = np.ascontiguousarray(
            np.asarray(Wo, np.float32)[cols, :]
            .reshape(DG // P, P, D)
            .transpose(1, 0, 2)
            .astype(BF16NP)
        )
        in_maps.append(
            {
                "xT": np.ascontiguousarray(x[b].T.astype(BF16NP)),
                "wq": wslice(Wq),
                "wk": wslice(Wk),
                "wv": wslice(Wv),
                "wo": wo_c,
            }
        )
    return in_maps


def kernel(x, Wq, Wk, Wv, Wo, bo):
    nc = build_nc()
    in_maps = make_in_maps(x, Wq, Wk, Wv, Wo)
    trace = bool(int(os.environ.get("MHA_TRACE", "0")))
    if trace:
        _install_ntff_hook()
    res = run_bass_kernel_spmd(
        nc, in_maps, core_ids=list(range(NCORES)), trace=trace,
        trace_cores=list(range(NCORES)) if trace else None,
    )
    _CACHE["last_results"] = res
    bo = np.asarray(bo, np.float32)
    out = np.zeros((B, S, D), np.float32)
    for c in range(NCORES):
        out[c // GROUPS] += np.asarray(res.results[c]["out"], dtype=np.float32)
    out += bo[None, None, :]
    return out


# revision 29
# speedup vs baseline: 1.4434x; 1.0693x over previous
"""Multi-head causal attention (B=2, S=2048, D=1024, H=16) on 8 trn2 cores.

Sharding: core c handles batch b = c // 4 and head group g = c % 4 (4 heads,
256 feature columns). Each core computes its heads' attention context and a
partial output projection (ctx_g @ Wo[rows_g]); the host sums the 4 partials
per batch (upcasting the bf16 partials) and adds bo.

v4 layout (all matmul operands bf16, fp32 PSUM accumulate):
- x is host-transposed to xT [D, S] bf16 so the QKV contraction dim sits on
  SBUF partitions; Q^T/K^T are produced head-major (head h on partitions
  [64*(h%2):+64] of tile m=h//2) so score matmuls contract 64 partitions with
  matching base partitions and no transposes; P^T = exp(S^T) feeds the PV
  matmul directly as the moving operand.
- Attention is sq-half-major (sq halves of 1024): per (head, sk-tile) one
  [128,1024] score psum (matmuls segmented at 512 -- PSUM bank limit), ONE
  wide exp (bf16 out), gpsimd affine_select on the diagonal block, PV
  accumulated into per-(head,half,512-seg) ctx psums with exact chain flags.
- Softmax denominator: ones column folded into V; normalize at the ctx
  psum->sbuf copy via DVE row copy + gpsimd partition_broadcast +
  reciprocal_approx_fast (no DRAM round trip, no slow full reciprocal);
  two-phase emission keeps the broadcast from stalling the gpsimd queue.
- PSUM: scores 2x[128,1024] (4 banks) + ctx 3x[128,512] (3) + filler
  1x[128,512] (1). The filler bank runs projection / out-projection chains
  dripped one per attention step, so the PE always has work while the scalar
  engine streams the ~80us of exps -- keeping the PE p-state at full clock.
"""

import os
import sys
import types
from contextlib import ExitStack

import numpy as np
import ml_dtypes

import concourse.bacc as bacc
import concourse.bass as bass
import concourse.mybir as mybir
import concourse.tile as tile
from concourse.bass_utils import run_bass_kernel_spmd


def _install_ntff_hook():
    """The agent image's antenv lacks axon_hooks, so trn_boot's NTFF hook
    install degrades silently. Recreate the module + hook so trace=True works."""
    if "antenv.axon_hooks" in sys.modules:
        return
    try:
        mod = types.ModuleType("antenv.axon_hooks")
        holder = [None]
        mod.set_axon_ntff_profile_hook = lambda h: holder.__setitem__(0, h)
        mod.get_axon_ntff_profile_hook = lambda: holder[0]
        from trn_agent_boot.trn_boot import _ntff_profile_via_ctypes

        hook = _ntff_profile_via_ctypes("/opt/axon/libaxon_pjrt.so")
        if hook is None:
            return
        mod.set_axon_ntff_profile_hook(hook)
        sys.modules["antenv.axon_hooks"] = mod
    except Exception:
        pass


B, S, D, H, HD = 2, 2048, 1024, 16, 64
NCORES = 8
GROUPS = 4          # head groups (cores) per batch
HC = H // GROUPS    # heads per core
DG = HC * HD        # feature columns per core (256)
P = 128
KSUB = D // P       # 8 contraction subtiles for the projections
SLC = 1024          # projection s-slice / attention half width
NST = S // P        # 16 sk subtiles of 128
F32 = mybir.dt.float32
BF16 = mybir.dt.bfloat16
BF16NP = ml_dtypes.bfloat16

_CACHE = {}


class _MHA:
    """Holds tile handles so emission helpers can be interleaved freely."""

    def __init__(self, tc, ctx, xT, wq, wk, wv, wo, out):
        self.tc = tc
        self.nc = tc.nc
        self.out = out
        self.scale = 1.0 / float(np.sqrt(np.float32(HD)))
        nc = self.nc

        self.consts = ctx.enter_context(tc.tile_pool(name="consts", bufs=1))
        self.sps = ctx.enter_context(tc.tile_pool(name="sps", bufs=2, space="PSUM"))
        self.cps = ctx.enter_context(tc.tile_pool(name="cps", bufs=3, space="PSUM"))
        self.fps = ctx.enter_context(tc.tile_pool(name="fps", bufs=1, space="PSUM"))
        self.xw = ctx.enter_context(tc.tile_pool(name="xw", bufs=2))
        self.ptp = ctx.enter_context(tc.tile_pool(name="ptp", bufs=4))
        self.smalls = ctx.enter_context(tc.tile_pool(name="smalls", bufs=4))
        self.bcp = ctx.enter_context(tc.tile_pool(name="bcp", bufs=3))
        self.outp = ctx.enter_context(tc.tile_pool(name="outp", bufs=3))

        c = self.consts
        self.qt = c.tile([P, 2, S], BF16)    # head h rows at [64*(h%2), h//2]
        self.kt = c.tile([P, 2, S], BF16)
        self.vt = c.tile([P, NST, HC, P], BF16)  # [sk, sst, h, 64v+1+63pad]
        self.ctxt = c.tile([P, 2, S], BF16)  # normalized ctx^T, qt layout
        self.wq_sb = c.tile([P, KSUB, DG], BF16)
        self.wk_sb = c.tile([P, KSUB, DG], BF16)
        self.wv_sb = c.tile([P, KSUB, DG], BF16)
        self.wo_sb = c.tile([P, DG // P, D], BF16)
        # issued in consumption order (wq before x before wk ...; wo last)
        nc.sync.dma_start(out=self.wq_sb, in_=wq)
        self._w_dmas = (wk, wv, wo)

        # V pad columns must be zero (psum garbage must stay finite); one-off
        # on gpsimd, off the DVE critical path.
        nc.gpsimd.memset(self.vt, 0.0)
        osc = c.tile([P, 1], F32)
        nc.vector.memset(osc, 1.0)
        # Warm the gpsimd custom-op library during the DMA lead-in: the first
        # PartitionBroadcast/affine_select otherwise pays a ~4us Q7 library
        # load in the middle of the kernel.
        warm = c.tile([P, 8], F32)
        nc.gpsimd.partition_broadcast(warm, osc[0:1, :].to_broadcast((1, 8)), channels=P)
        nc.gpsimd.affine_select(
            out=warm, in_=warm, pattern=[[1, 8]],
            compare_op=mybir.AluOpType.is_ge, fill=0.0, base=0,
            channel_multiplier=-1,
        )
        for h in range(HC):
            ones_col = 64 if h % 2 == 0 else 0
            nc.vector.tensor_copy(
                out=self.vt[:, :, h, ones_col : ones_col + 1],
                in_=osc[:, None, :].to_broadcast((P, NST, 1)),
            )

        self.xT = xT
        self.xn = [None, None]
        self.cpsum = {}  # (h, half, seg) -> [128,512] psum tile
        self.pending_norm = []

    # ---- projections (filler units on the 1-bank fps pool) ------------
    def dma_x(self, n):
        xn = self.xw.tile([P, KSUB, SLC], BF16, tag="xn", name=f"xn_{n}")
        for k in range(KSUB):
            self.nc.sync.dma_start(
                out=xn[:, k, :], in_=self.xT[k * P : (k + 1) * P, n * SLC : (n + 1) * SLC]
            )
        self.xn[n] = xn

    def proj_qk(self, n, m, which, half, ps=None):
        """One [128,512] chain of Q^T or K^T: slice n, head-pair m, 512-half.
        `ps` lets the lead-in pass its own psum slice so the four lead chains
        run back-to-back (no copy-wait) and the PE p-state ramps."""
        nc = self.nc
        w = self.wq_sb if which == "q" else self.wk_sb
        dst = self.qt if which == "q" else self.kt
        if ps is None:
            ps = self.fps.tile([P, 512], F32, tag="f", name=f"{which}ps_{n}_{m}_{half}")
        for k in range(KSUB):
            nc.tensor.matmul(
                ps,
                lhsT=w[:, k, m * P : (m + 1) * P],
                rhs=self.xn[n][:, k, half * 512 : (half + 1) * 512],
                start=(k == 0),
                stop=(k == KSUB - 1),
            )
        c0 = n * SLC + half * 512
        nc.vector.tensor_copy(out=dst[:, m, c0 : c0 + 512], in_=ps)

    def proj_v(self, n, j2):
        """V for s-subtiles (8n+2*j2, 8n+2*j2+1) -> vt rows with ones layout."""
        nc = self.nc
        ps = self.fps.tile([P, 512], F32, tag="f", name=f"vps_{n}_{j2}")
        for j in range(2):
            for k in range(KSUB):
                nc.tensor.matmul(
                    ps[:, j * DG : (j + 1) * DG],
                    lhsT=self.xn[n][:, k, (2 * j2 + j) * P : (2 * j2 + j + 1) * P],
                    rhs=self.wv_sb[:, k, :],
                    start=(k == 0),
                    stop=(k == KSUB - 1),
                )
        st0 = 8 * n + 2 * j2
        psv = ps.rearrange("p (t h d) -> p t h d", h=HC, d=HD)
        nc.vector.tensor_copy(
            out=self.vt[:, st0 : st0 + 2, 0:HC:2, 0:HD], in_=psv[:, :, 0:HC:2, :]
        )
        nc.vector.tensor_copy(
            out=self.vt[:, st0 : st0 + 2, 1:HC:2, HD:P], in_=psv[:, :, 1:HC:2, :]
        )

    # ---- attention ----------------------------------------------------
    def att_step(self, h, half, ski, fill_cb=None):
        """One (head, sk-tile) step in an sq half: segmented score matmuls,
        one wide exp, diagonal mask, segmented PV accumulate. fill_cb (if
        given) is emitted between the exp and the PV matmuls, so filler PE
        work lands where the PE would otherwise wait on the exp."""
        nc = self.nc
        hp = 64 * (h % 2)
        hm = h // 2
        g0 = half * SLC            # global sq start of this half
        a = max(g0, P * ski)       # global sq start of the valid window
        w = g0 + SLC - a
        if w <= 0:
            return
        diag = P * ski >= g0
        l0 = a - g0
        segs = [s for s in ((l0, min(512, SLC)), (max(l0, 512), SLC)) if s[0] < s[1]]

        spsum = self.sps.tile([P, SLC], F32, tag="ps", name=f"s_{h}_{half}_{ski}")
        for s0, s1 in segs:
            nc.tensor.matmul(
                spsum[:, s0:s1],
                lhsT=self.kt[hp : hp + 64, hm, ski * P : (ski + 1) * P],
                rhs=self.qt[hp : hp + 64, hm, g0 + s0 : g0 + s1],
                start=True,
                stop=True,
            )
        pt = self.ptp.tile([P, SLC], BF16, tag="pt", name=f"pt_{h}_{half}_{ski}")
        nc.scalar.activation(
            out=pt[:, 0:w],
            in_=spsum[:, l0:SLC],
            func=mybir.ActivationFunctionType.Exp,
            bias=0.0,
            scale=float(self.scale),
        )
        if diag:
            nc.gpsimd.affine_select(
                out=pt[:, 0:P],
                in_=pt[:, 0:P],
                pattern=[[1, P]],
                compare_op=mybir.AluOpType.is_ge,
                fill=0.0,
                base=0,
                channel_multiplier=-1,
            )
        if fill_cb is not None:
            fill_cb()
        for s0, s1 in segs:
            seg = s0 >= 512
            sqt = 2 * half + seg
            key = (h, half, seg)
            if key not in self.cpsum:
                self.cpsum[key] = self.cps.tile(
                    [P, 512], F32, tag="ctx", name=f"ctx_{h}_{half}_{seg}"
                )
            nc.tensor.matmul(
                self.cpsum[key][:, s0 - 512 * seg : s1 - 512 * seg],
                lhsT=self.vt[:, ski, h, :],
                rhs=pt[:, s0 - l0 : s1 - l0],
                start=(ski == 0),
                stop=(ski == 4 * sqt + 3),
            )
        if ski % 4 == 3:
            sqt = ski // 4
            if sqt in (2 * half, 2 * half + 1):
                self.norm_start(h, half, sqt)

    def norm_start(self, h, half, sqt):
        """Phase 1 of the eager normalize: pull the denominator row out of
        the finished ctx psum. Phase 2 is flushed later (one sq-tile lag)."""
        nc = self.nc
        denom_row = 64 if h % 2 == 0 else 0
        cpsum = self.cpsum[(h, half, sqt % 2)]
        den = self.smalls.tile([1, 512], F32, tag="den", name=f"den_{h}_{sqt}")
        nc.vector.tensor_copy(out=den, in_=cpsum[denom_row : denom_row + 1, :])
        self.pending_norm.append((h, half, sqt, den))

    def norm_flush(self):
        nc = self.nc
        for h, half, sqt, den in self.pending_norm:
            hp = 64 * (h % 2)
            hm = h // 2
            ctx_rows = 0 if h % 2 == 0 else 64
            cpsum = self.cpsum[(h, half, sqt % 2)]
            sq0 = sqt * 512
            bc = self.bcp.tile([P, 512], F32, tag="bc", name=f"bc_{h}_{sqt}")
            nc.gpsimd.partition_broadcast(bc, den, channels=P)
            nc.vector.reciprocal_approx_fast(out=bc, in_=bc)
            nc.vector.tensor_tensor(
                self.ctxt[hp : hp + 64, hm, sq0 : sq0 + 512],
                cpsum[ctx_rows : ctx_rows + 64, :],
                bc[ctx_rows : ctx_rows + 64, :],
                mybir.AluOpType.mult,
            )
        self.pending_norm = []

    # ---- output projection (filler units) -----------------------------
    def outproj(self, st, nn, pool=None):
        """pool='ctx' uses the (freed) ctx psum rotation -- for the tail,
        where the 3-deep rotation hides the psum->sbuf copies; the default
        1-bank filler pool serializes unit-to-unit."""
        nc = self.nc
        if pool == "ctx":
            ps = self.cps.tile([P, 512], F32, tag="ctx", name=f"ops_{st}_{nn}")
        else:
            ps = self.fps.tile([P, 512], F32, tag="f", name=f"ops_{st}_{nn}")
        for k in range(DG // P):
            nc.tensor.matmul(
                ps,
                lhsT=self.ctxt[:, k, st * P : (st + 1) * P],
                rhs=self.wo_sb[:, k, nn * 512 : (nn + 1) * 512],
                start=(k == 0),
                stop=(k == DG // P - 1),
            )
        ot = self.outp.tile([P, 512], BF16, tag="ot", name=f"ot_{st}_{nn}")
        if pool == "ctx" and (st + nn) % 2 == 0:
            # tail: ACT is idle -- split the psum->sbuf copies across engines
            nc.scalar.copy(out=ot, in_=ps)
        else:
            nc.vector.tensor_copy(out=ot, in_=ps)
        nc.sync.dma_start(
            out=self.out[st * P : (st + 1) * P, nn * 512 : (nn + 1) * 512], in_=ot
        )


def _emit(tc, xT, wq, wk, wv, wo, out):
    with ExitStack() as ctx:
        m = _MHA(tc, ctx, xT, wq, wk, wv, wo, out)

        # Minimal lead-in: wq is already in flight; x slice 0, then the
        # other weights behind it, then only the Q/K tiles head 0 needs.
        m.dma_x(0)
        wk_d, wv_d, wo_d = m._w_dmas
        m.nc.sync.dma_start(out=m.wk_sb, in_=wk_d)
        m.dma_x(1)
        m.nc.sync.dma_start(out=m.wv_sb, in_=wv_d)
        m.nc.sync.dma_start(out=m.wo_sb, in_=wo_d)
        # four back-to-back chains across three psum homes (no copy-waits)
        lead_ps = m.sps.tile([P, SLC], F32, tag="ps", name="lead_ps")
        m.proj_qk(0, 0, "q", 0)
        m.proj_qk(0, 0, "k", 0, ps=lead_ps[:, 0:512])
        m.proj_qk(0, 0, "q", 1, ps=lead_ps[:, 512:1024])
        m.proj_qk(0, 0, "k", 1)

        # Remaining projection work becomes filler units, dripped one per
        # attention step so the PE never idles while ACT streams the exps.
        # V units for slice 0 MUST be emitted before the h0 att_step that
        # first consumes them (PV of ski needs vt s-subtile ski).
        fill = []
        fill += [lambda hf=hf: m.proj_qk(0, 1, "q", hf) for hf in range(2)]
        fill += [lambda hf=hf: m.proj_qk(0, 1, "k", hf) for hf in range(2)]
        fill += [lambda hf=hf: m.proj_qk(1, 0, "q", hf) for hf in range(2)]
        fill += [lambda hf=hf: m.proj_qk(1, 0, "k", hf) for hf in range(2)]
        fill += [lambda j2=j2: m.proj_v(1, j2) for j2 in range(4)]
        fill += [lambda hf=hf: m.proj_qk(1, 1, "q", hf) for hf in range(2)]
        fill += [lambda hf=hf: m.proj_qk(1, 1, "k", hf) for hf in range(2)]
        fi = 0

        # half 0: heads 0..3 over ski 0..7. h0's V units are emitted inside
        # the att_step (between exp and PV) so the first exps aren't delayed,
        # while still preceding the PV matmuls that consume them.
        for h in range(HC):
            for ski in range(8):
                cb = None
                if h == 0 and ski % 2 == 0:
                    cb = (lambda j2=ski // 2: m.proj_v(0, j2))
                m.att_step(h, 0, ski, fill_cb=cb)
                if h >= 1 and fi < len(fill):
                    fill[fi](); fi += 1
                if ski % 4 == 1:
                    m.norm_flush()
            m.norm_flush()
        while fi < len(fill):
            fill[fi](); fi += 1

        # half 1: heads 0..3 over ski 0..15; out-projection of the finished
        # first half (st 0..7) is the filler stream, spread thinly so the PE
        # never idles long enough to drop its p-state. st 8..11 unlock once
        # h3's sq-tile-2 normalize is flushed (h3 ski 13).
        ops = [(st, nn) for st in range(8) for nn in range(2)]
        oi = 0
        step = 0
        for h in range(HC):
            for ski in range(16):
                m.att_step(h, 1, ski)
                step += 1
                if step % 3 == 0 and oi < len(ops):
                    st, nn = ops[oi]; m.outproj(st, nn); oi += 1
                if ski % 4 == 1:
                    m.norm_flush()
                if h == 3 and ski == 14:
                    # h3's sq-tile-2 normalize flushed at ski 13: st8 can go
                    # on the two free ctx psum slots
                    m.outproj(8, 0, pool="ctx")
                    m.outproj(8, 1, pool="ctx")
            m.norm_flush()
        while oi < len(ops):
            st, nn = ops[oi]; m.outproj(st, nn); oi += 1

        # tail on the freed ctx psum rotation (hides the psum->sbuf copies)
        for st in range(9, NST):
            for nn in range(2):
                m.outproj(st, nn, pool="ctx")


def build_nc():
    if "nc" in _CACHE:
        return _CACHE["nc"]
    nc = bacc.Bacc("TRN2", target_bir_lowering=False, debug=False, num_devices=NCORES)
    xT = nc.dram_tensor("xT", (D, S), BF16, kind="ExternalInput").ap()
    wq = nc.dram_tensor("wq", (P, KSUB, DG), BF16, kind="ExternalInput").ap()
    wk = nc.dram_tensor("wk", (P, KSUB, DG), BF16, kind="ExternalInput").ap()
    wv = nc.dram_tensor("wv", (P, KSUB, DG), BF16, kind="ExternalInput").ap()
    wo = nc.dram_tensor("wo", (P, DG // P, D), BF16, kind="ExternalInput").ap()
    out = nc.dram_tensor("out", (S, D), BF16, kind="ExternalOutput").ap()
    with tile.TileContext(nc) as tc:
        _emit(tc, xT, wq, wk, wv, wo, out)
    nc.compile()
    _CACHE["nc"] = nc
    return nc


def make_in_maps(x, Wq, Wk, Wv, Wo):
    x = np.asarray(x, np.float32)
    in_maps = []
    for c in range(NCORES):
        b, g = c // GROUPS, c % GROUPS
        cols = slice(g * DG, (g + 1) * DG)

        def wslice(W):
            # [D, DG] -> [128, KSUB, DG] with [p, k, m] = W[k*128+p, m]
            return np.ascontiguousarray(
                np.asarray(W, np.float32)[:, cols]
                .reshape(KSUB, P, DG)
                .transpose(1, 0, 2)
                .astype(BF16NP)
            )

        wo_c = np.ascontiguousarray(
            np.asarray(Wo, np.float32)[cols, :]
            .reshape(DG // P, P, D)
            .transpose(1, 0, 2)
            .astype(BF16NP)
        )
        in_maps.append(
            {
                "xT": np.ascontiguousarray(x[b].T.astype(BF16NP)),
                "wq": wslice(Wq),
                "wk": wslice(Wk),
                "wv": wslice(Wv),
                "wo": wo_c,
            }
        )
    return in_maps


def kernel(x, Wq, Wk, Wv, Wo, bo):
    nc = build_nc()
    in_maps = make_in_maps(x, Wq, Wk, Wv, Wo)
    trace = bool(int(os.environ.get("MHA_TRACE", "0")))
    if trace:
        _install_ntff_hook()
    res = run_bass_kernel_spmd(
        nc, in_maps, core_ids=list(range(NCORES)), trace=trace,
        trace_cores=list(range(NCORES)) if trace else None,
    )
    _CACHE["last_results"] = res
    bo = np.asarray(bo, np.float32)
    out = np.zeros((B, S, D), np.float32)
    for c in range(NCORES):
        out[c // GROUPS] += np.asarray(res.results[c]["out"], dtype=np.float32)
    out += bo[None, None, :]
    return out
